# revision 13
# baseline (speedup 1.0000x reference)
"""Trainium2 Bass kernel for pointer-generator final-distribution (scatter_memory).

out[r, v] = p_gens[r] * vocab_ds[r, v]  (+ (1-p_gens[r])*attns[r, l_win]  at
v == sources[l, b(r)], duplicate source ids resolved last-occurrence-wins)

Strategy (8 NeuronCores, SPMD), bf16 streaming (DMA/HBM-bound):
  - Pure memory streaming: every element is read once and written once.
    Per-core HBM limit is ~358 GB/s, so bytes are the only lever. rel-err
    tolerance is 2e-2; bf16 (~1% err) halves traffic vs f32: host uploads
    vocab as bf16, device computes/stores bf16, host upconverts.
  - Shard by batch column: core k owns b in {4k..4k+3}, all T decoder steps
    (rows r = t*B + b). Host pre-gathers rows b-major so device DMAs are
    contiguous; two 128-row groups per core (2 b's x 64 t each).
  - One engine, one job (no cross-engine semaphore stalls in a stream):
    SP(sync) issues all loads, ACT does the per-partition p-scale,
    PE matmuls host-baked one-hots into PSUM (scatter projection),
    DVE merges PSUM into the tile, GPSIMD issues stores via SWDGE.
  - Scatter: for each 1024-wide subtile, host bakes [K, 128] bf16 update
    values (block-diagonal over the two b's) and the one-hot [K, 1024]
    selector (is_equal was measured to cost 92us of DVE - host-baking it
    is ~6 MB of extra DMA instead). PE computes vals.T @ onehot -> PSUM;
    DVE adds 2048-wide PSUM chunks into the streamed tile (PSUM operands
    run DVE at 1x, so coalescing to 2048 amortizes instr overhead).
"""

import numpy as np

N_CORES = 8
WIN = 8192
SUB = 512
KROW = 16


def _host_prep(vocab_ds, attns, p_gens, sources, T):
    import ml_dtypes
    f32 = np.float32
    bf16 = ml_dtypes.bfloat16
    vocab_ds = np.ascontiguousarray(vocab_ds, dtype=f32)
    attns = np.ascontiguousarray(attns, dtype=f32)
    p_gens = np.ascontiguousarray(p_gens, dtype=f32)
    src = np.asarray(sources).astype(np.int64)
    rows, V = vocab_ds.shape
    L, B = src.shape
    assert rows == T * B

    ag = (f32(1.0) - p_gens) * attns  # gated copy dist, [rows, L]

    # winners per batch column: duplicate source ids -> last occurrence wins
    wins = []
    for b in range(B):
        d = {}
        col = src[:, b]
        for l in range(L):
            d[int(col[l])] = l
        cols = np.fromiter(d.keys(), dtype=np.int64)
        ls = np.fromiter(d.values(), dtype=np.int64)
        o = np.argsort(cols)
        wins.append((cols[o], ls[o]))

    # global 512-col subtile grid
    NS = (V + SUB - 1) // SUB
    sub_geom = [(i * SUB, min(SUB, V - i * SUB)) for i in range(NS)]  # (c0, wd)

    # per-group window tables; the tail of the LAST-processed group is cut
    # into 2048-col units so the pipeline drains in small steps
    def windows_for(split_tail):
        wt = []
        c = 0
        while c < V:
            if split_tail and V - c <= 10240 + WIN - 8192 and V - c > WIN // 4:
                ww = min(WIN // 4, V - c)
            else:
                ww = min(WIN, V - c)
            if V - c - ww < SUB and V - c - ww > 0:
                ww = V - c  # never leave a sub-SUB sliver
            wt.append((c, ww))
            c += ww
        return wt

    BPC = B // N_CORES  # 4
    G = BPC // 2        # 2 groups of 2 b's
    wins_tbl = [windows_for(g == G - 1) for g in range(G)]

    # bucket updates per (core, g, subtile)
    upd = [[[[] for _ in range(NS)] for _ in range(G)] for _ in range(N_CORES)]
    for core in range(N_CORES):
        for g in range(G):
            for half in range(2):
                b = core * BPC + g * 2 + half
                cols, ls = wins[b]
                for c, l in zip(cols.tolist(), ls.tolist()):
                    upd[core][g][c // SUB].append((half, c, l))

    # uniform-per-(g, subtile) K across cores; >=1 so every PSUM region is
    # written (a zero one-hot row yields zeros after start=True reset)
    K_ws = [[max(1, max(len(upd[core][g][i]) for core in range(N_CORES)))
             for i in range(NS)] for g in range(G)]
    assert all(k <= 2 * KROW for g in range(G) for k in K_ws[g]), \
        "subtile update count exceeds 2*KROW"
    # Every scatter job is a [KROW, 128+SUB] block (vals | one-hot); rows
    # beyond the real K are zero (numerically exact). Subtiles with
    # K > KROW get a second accumulate job (start=False). Exactly KROW
    # rows per DMA keeps the row->SDMA-engine round-robin balanced.
    # jobs[g][w] = list of (jidx, s_local, wd, start, stop, i, klo)
    jobs = [[[] for _ in range(len(wins_tbl[g]))] for g in range(G)]
    win_info = []  # per (g,w): (i0, nsub, j0, njobs)
    NJ = []
    for g in range(G):
        wi = []
        jidx = 0
        for w, (c0w, ww) in enumerate(wins_tbl[g]):
            i0 = c0w // SUB
            nsub = (ww + SUB - 1) // SUB
            j0 = jidx
            for s in range(nsub):
                i = i0 + s
                K = K_ws[g][i]
                wd = sub_geom[i][1]
                if K <= KROW:
                    jobs[g][w].append((jidx, s, wd, True, True, i, 0))
                    jidx += 1
                else:
                    jobs[g][w].append((jidx, s, wd, True, False, i, 0))
                    jobs[g][w].append((jidx + 1, s, wd, False, True, i, KROW))
                    jidx += 2
            wi.append((i0, nsub, j0, jidx - j0))
        win_info.append(wi)
        NJ.append(jidx)

    # per-core device inputs
    in_maps = []
    for core in range(N_CORES):
        m = {}
        for g in range(G):
            row_idx = []
            for half in range(2):
                b = core * BPC + g * 2 + half
                row_idx.extend(t * B + b for t in range(T))
            row_idx = np.asarray(row_idx)
            m[f"vocab{g}"] = vocab_ds[row_idx].astype(bf16)
            m[f"pgen{g}"] = p_gens[row_idx]
            # merged per-job [KROW, 128 vals | 512 one-hot] blocks so the
            # scatter operands ship as one balanced DMA stream
            W = 128 + SUB
            ohv = np.zeros((KROW, NJ[g] * W), dtype=f32)
            for w in range(len(wins_tbl[g])):
                for (jj, s, wd, st, sp, i, klo) in jobs[g][w]:
                    c0 = sub_geom[i][0]
                    ups = upd[core][g][i][klo:klo + KROW]
                    for k, (half, c, l) in enumerate(ups):
                        r0 = half * T
                        ohv[k, jj * W + r0: jj * W + r0 + T] = \
                            ag[row_idx[r0: r0 + T], l]
                        ohv[k, jj * W + 128 + (c - c0)] = 1.0
            m[f"ohv{g}"] = ohv.astype(bf16)
        in_maps.append(m)

    meta = dict(V=V, T=T, B=B, NS=NS, G=G, sub_geom=sub_geom,
                K_ws=K_ws, BPC=BPC, win_info=win_info,
                jobs=jobs, NJ=NJ, wins_tbl=wins_tbl)
    return in_maps, meta


def _build_nc(meta):
    from concourse import bacc, mybir

    V, NS, G = meta["V"], meta["NS"], meta["G"]
    sub_geom, K_ws = meta["sub_geom"], meta["K_ws"]
    wins_tbl = meta["wins_tbl"]
    f32 = mybir.dt.float32

    bf16 = mybir.dt.bfloat16
    nc = bacc.Bacc(None, target_bir_lowering=False, debug=False)
    vocab = [nc.declare_dram_parameter(f"vocab{g}", [128, V], bf16, isOutput=False)
             for g in range(G)]
    pgen = [nc.declare_dram_parameter(f"pgen{g}", [128, 1], f32, isOutput=False)
            for g in range(G)]
    SW = 128 + SUB
    NJ = meta["NJ"]
    ohv = [nc.declare_dram_parameter(f"ohv{g}", [KROW, NJ[g] * SW], bf16, isOutput=False)
           for g in range(G)]
    out = [nc.declare_dram_parameter(f"out{g}", [128, V], bf16, isOutput=True)
           for g in range(G)]

    from concourse.tile import TileContext

    win_info = meta["win_info"]
    jobs = meta["jobs"]
    # SBUF base partition alternates {0,64} per window so the narrow
    # (KROW-partition) ohv DMAs spread over both SDMA engine halves
    # (matmul requires base partition in {0,32,64}, lhsT/rhs bases equal)
    SW = 128 + SUB
    with TileContext(nc) as tc:
        with tc.tile_pool(name="io", bufs=7) as io_pool, \
             tc.tile_pool(name="small", bufs=1) as small, \
             tc.tile_pool(name="oh", bufs=3) as oh_pool, \
             tc.tile_pool(name="psum", bufs=2, space="PSUM") as psum_pool:

            p_t = []
            for g in range(G):
                p_t.append(small.tile([128, 1], f32, tag=f"p{g}", name=f"p{g}"))

            for g in range(G):
                nc.sync.dma_start(out=p_t[g][:], in_=pgen[g][:])
                for w, (c0w, ww) in enumerate(wins_tbl[g]):
                    i0, nsub, j0, njobs = win_info[g][w]
                    oo = 64 * ((w + g) % 2)
                    t = io_pool.tile([128, WIN], bf16, tag="io")
                    nc.sync.dma_start(out=t[:, :ww], in_=vocab[g][:, c0w:c0w + ww])
                    oh_t = oh_pool.tile([128, 18 * SW], bf16,
                                        tag="oh", name="oht")
                    nc.sync.dma_start(
                        out=oh_t[oo:oo + KROW, :njobs * SW],
                        in_=ohv[g][:, j0 * SW:(j0 + njobs) * SW])
                    # PSUM chunks of 4 subtiles (2048 cols); one fused DVE
                    # pass per chunk does scale+merge: t = t*p + ps
                    wjobs = jobs[g][w]
                    for s0 in range(0, nsub, 4):
                        ns = min(4, nsub - s0)
                        ck_lo = s0 * SUB
                        ck_w = sum(sub_geom[i0 + s0 + j][1] for j in range(ns))
                        ps = psum_pool.tile([128, 4 * SUB], f32, tag="ps", name="ps")
                        for (jj, s, wd, st, sp, i, klo) in wjobs:
                            if not (s0 <= s < s0 + ns):
                                continue
                            sl = (jj - j0) * SW
                            nc.tensor.matmul(
                                out=ps[:, (s - s0) * SUB:(s - s0) * SUB + wd],
                                lhsT=oh_t[oo:oo + KROW, sl:sl + 128],
                                rhs=oh_t[oo:oo + KROW, sl + 128:sl + 128 + wd],
                                start=st, stop=sp)
                        nc.vector.scalar_tensor_tensor(
                            out=t[:, ck_lo:ck_lo + ck_w],
                            in0=t[:, ck_lo:ck_lo + ck_w],
                            scalar=p_t[g][:, :1],
                            in1=ps[:, :ck_w],
                            op0=mybir.AluOpType.mult,
                            op1=mybir.AluOpType.add)
                    nc.scalar.dma_start(out=out[g][:, c0w:c0w + ww], in_=t[:, :ww])
    nc.finalize()
    return nc


def kernel(vocab_ds, attns, p_gens, sources, decoder_batch_len):
    T = int(decoder_batch_len)
    in_maps, meta = _host_prep(vocab_ds, attns, p_gens, sources, T)
    nc = _build_nc(meta)

    from concourse.bass_utils import run_bass_kernel_spmd
    res = run_bass_kernel_spmd(nc, in_maps, list(range(N_CORES)))

    rows, V = np.asarray(vocab_ds).shape
    B, BPC, G = meta["B"], meta["BPC"], meta["G"]
    full = np.empty((rows, V), dtype=np.float32)
    for core in range(N_CORES):
        for g in range(G):
            blk = np.asarray(res.results[core][f"out{g}"], dtype=np.float32)
            for half in range(2):
                b = core * BPC + g * 2 + half
                full[b::B] = blk[half * T:(half + 1) * T]
    return full


# revision 14
# speedup vs baseline: 1.0027x; 1.0027x over previous
"""Trainium2 Bass kernel for pointer-generator final-distribution (scatter_memory).

out[r, v] = p_gens[r] * vocab_ds[r, v]  (+ (1-p_gens[r])*attns[r, l_win]  at
v == sources[l, b(r)], duplicate source ids resolved last-occurrence-wins)

Strategy (8 NeuronCores, SPMD), bf16 streaming (DMA/HBM-bound):
  - Pure memory streaming: every element is read once and written once.
    Per-core HBM limit is ~358 GB/s, so bytes are the only lever. rel-err
    tolerance is 2e-2; bf16 (~1% err) halves traffic vs f32: host uploads
    vocab as bf16, device computes/stores bf16, host upconverts.
  - Shard by batch column: core k owns b in {4k..4k+3}, all T decoder steps
    (rows r = t*B + b). Host pre-gathers rows b-major so device DMAs are
    contiguous; two 128-row groups per core (2 b's x 64 t each).
  - One engine, one job (no cross-engine semaphore stalls in a stream):
    SP(sync) issues all loads, ACT does the per-partition p-scale,
    PE matmuls host-baked one-hots into PSUM (scatter projection),
    DVE merges PSUM into the tile, GPSIMD issues stores via SWDGE.
  - Scatter: for each 1024-wide subtile, host bakes [K, 128] bf16 update
    values (block-diagonal over the two b's) and the one-hot [K, 1024]
    selector (is_equal was measured to cost 92us of DVE - host-baking it
    is ~6 MB of extra DMA instead). PE computes vals.T @ onehot -> PSUM;
    DVE adds 2048-wide PSUM chunks into the streamed tile (PSUM operands
    run DVE at 1x, so coalescing to 2048 amortizes instr overhead).
"""

import numpy as np

N_CORES = 8
WIN = 8192
SUB = 512
KROW = 16


def _host_prep(vocab_ds, attns, p_gens, sources, T):
    import ml_dtypes
    f32 = np.float32
    bf16 = ml_dtypes.bfloat16
    vocab_ds = np.ascontiguousarray(vocab_ds, dtype=f32)
    attns = np.ascontiguousarray(attns, dtype=f32)
    p_gens = np.ascontiguousarray(p_gens, dtype=f32)
    src = np.asarray(sources).astype(np.int64)
    rows, V = vocab_ds.shape
    L, B = src.shape
    assert rows == T * B

    ag = (f32(1.0) - p_gens) * attns  # gated copy dist, [rows, L]

    # winners per batch column: duplicate source ids -> last occurrence wins
    wins = []
    for b in range(B):
        d = {}
        col = src[:, b]
        for l in range(L):
            d[int(col[l])] = l
        cols = np.fromiter(d.keys(), dtype=np.int64)
        ls = np.fromiter(d.values(), dtype=np.int64)
        o = np.argsort(cols)
        wins.append((cols[o], ls[o]))

    # global 512-col subtile grid
    NS = (V + SUB - 1) // SUB
    sub_geom = [(i * SUB, min(SUB, V - i * SUB)) for i in range(NS)]  # (c0, wd)

    # per-group window tables; the tail of the LAST-processed group is cut
    # into 2048-col units so the pipeline drains in small steps
    def windows_for(split_tail):
        wt = []
        c = 0
        while c < V:
            if split_tail and V - c <= 10240 + WIN - 8192 and V - c > WIN // 4:
                ww = min(WIN // 4, V - c)
            else:
                ww = min(WIN, V - c)
            if V - c - ww < SUB and V - c - ww > 0:
                ww = V - c  # never leave a sub-SUB sliver
            wt.append((c, ww))
            c += ww
        return wt

    BPC = B // N_CORES  # 4
    G = BPC // 2        # 2 groups of 2 b's
    wins_tbl = [windows_for(g == G - 1) for g in range(G)]

    # bucket updates per (core, g, subtile)
    upd = [[[[] for _ in range(NS)] for _ in range(G)] for _ in range(N_CORES)]
    for core in range(N_CORES):
        for g in range(G):
            for half in range(2):
                b = core * BPC + g * 2 + half
                cols, ls = wins[b]
                for c, l in zip(cols.tolist(), ls.tolist()):
                    upd[core][g][c // SUB].append((half, c, l))

    # uniform-per-(g, subtile) K across cores; >=1 so every PSUM region is
    # written (a zero one-hot row yields zeros after start=True reset)
    K_ws = [[max(1, max(len(upd[core][g][i]) for core in range(N_CORES)))
             for i in range(NS)] for g in range(G)]
    assert all(k <= 2 * KROW for g in range(G) for k in K_ws[g]), \
        "subtile update count exceeds 2*KROW"
    # Every scatter job is a [KROW, 128+SUB] block (vals | one-hot); rows
    # beyond the real K are zero (numerically exact). Subtiles with
    # K > KROW get a second accumulate job (start=False). Exactly KROW
    # rows per DMA keeps the row->SDMA-engine round-robin balanced.
    # jobs[g][w] = list of (jidx, s_local, wd, start, stop, i, klo)
    jobs = [[[] for _ in range(len(wins_tbl[g]))] for g in range(G)]
    win_info = []  # per (g,w): (i0, nsub, j0, njobs)
    NJ = []
    for g in range(G):
        wi = []
        jidx = 0
        for w, (c0w, ww) in enumerate(wins_tbl[g]):
            i0 = c0w // SUB
            nsub = (ww + SUB - 1) // SUB
            j0 = jidx
            for s in range(nsub):
                i = i0 + s
                K = K_ws[g][i]
                wd = sub_geom[i][1]
                if K <= KROW:
                    jobs[g][w].append((jidx, s, wd, True, True, i, 0))
                    jidx += 1
                else:
                    jobs[g][w].append((jidx, s, wd, True, False, i, 0))
                    jobs[g][w].append((jidx + 1, s, wd, False, True, i, KROW))
                    jidx += 2
            wi.append((i0, nsub, j0, jidx - j0))
        win_info.append(wi)
        NJ.append(jidx)

    # per-core device inputs
    in_maps = []
    for core in range(N_CORES):
        m = {}
        for g in range(G):
            row_idx = []
            for half in range(2):
                b = core * BPC + g * 2 + half
                row_idx.extend(t * B + b for t in range(T))
            row_idx = np.asarray(row_idx)
            m[f"vocab{g}"] = vocab_ds[row_idx].astype(bf16)
            m[f"pgen{g}"] = p_gens[row_idx]
            # per-job [KROW, 128] bf16 vals + [KROW, 512] fp8 one-hot
            # (0/1 exact in fp8; halves the one-hot DMA bytes)
            fp8 = ml_dtypes.float8_e4m3
            vals = np.zeros((KROW, NJ[g] * 128), dtype=f32)
            oh = np.zeros((KROW, NJ[g] * SUB), dtype=fp8)
            for w in range(len(wins_tbl[g])):
                for (jj, s, wd, st, sp, i, klo) in jobs[g][w]:
                    c0 = sub_geom[i][0]
                    ups = upd[core][g][i][klo:klo + KROW]
                    for k, (half, c, l) in enumerate(ups):
                        r0 = half * T
                        vals[k, jj * 128 + r0: jj * 128 + r0 + T] = \
                            ag[row_idx[r0: r0 + T], l]
                        oh[k, jj * SUB + (c - c0)] = fp8(1.0)
            m[f"vals{g}"] = vals.astype(bf16)
            m[f"oh{g}"] = oh
        in_maps.append(m)

    meta = dict(V=V, T=T, B=B, NS=NS, G=G, sub_geom=sub_geom,
                K_ws=K_ws, BPC=BPC, win_info=win_info,
                jobs=jobs, NJ=NJ, wins_tbl=wins_tbl)
    return in_maps, meta


def _build_nc(meta):
    from concourse import bacc, mybir

    V, NS, G = meta["V"], meta["NS"], meta["G"]
    sub_geom, K_ws = meta["sub_geom"], meta["K_ws"]
    wins_tbl = meta["wins_tbl"]
    f32 = mybir.dt.float32

    bf16 = mybir.dt.bfloat16
    nc = bacc.Bacc(None, target_bir_lowering=False, debug=False)
    vocab = [nc.declare_dram_parameter(f"vocab{g}", [128, V], bf16, isOutput=False)
             for g in range(G)]
    pgen = [nc.declare_dram_parameter(f"pgen{g}", [128, 1], f32, isOutput=False)
            for g in range(G)]
    fp8 = mybir.dt.float8e4
    NJ = meta["NJ"]
    valsp = [nc.declare_dram_parameter(f"vals{g}", [KROW, NJ[g] * 128], bf16,
                                       isOutput=False) for g in range(G)]
    ohp = [nc.declare_dram_parameter(f"oh{g}", [KROW, NJ[g] * SUB], fp8,
                                     isOutput=False) for g in range(G)]
    out = [nc.declare_dram_parameter(f"out{g}", [128, V], bf16, isOutput=True)
           for g in range(G)]

    from concourse.tile import TileContext

    win_info = meta["win_info"]
    jobs = meta["jobs"]
    # SBUF base partition alternates {0,64} per window so the narrow
    # (KROW-partition) vals/oh DMAs spread over both SDMA engine halves
    # (matmul requires base partition in {0,32,64}, lhsT/rhs bases equal)
    with TileContext(nc) as tc:
        with tc.tile_pool(name="io", bufs=7) as io_pool, \
             tc.tile_pool(name="small", bufs=1) as small, \
             tc.tile_pool(name="oh", bufs=3) as oh_pool, \
             tc.tile_pool(name="psum", bufs=2, space="PSUM") as psum_pool:

            p_t = []
            for g in range(G):
                p_t.append(small.tile([128, 1], f32, tag=f"p{g}", name=f"p{g}"))

            for g in range(G):
                nc.sync.dma_start(out=p_t[g][:], in_=pgen[g][:])
                for w, (c0w, ww) in enumerate(wins_tbl[g]):
                    i0, nsub, j0, njobs = win_info[g][w]
                    oo = 64 * ((w + g) % 2)
                    t = io_pool.tile([128, WIN], bf16, tag="io")
                    nc.sync.dma_start(out=t[:, :ww], in_=vocab[g][:, c0w:c0w + ww])
                    vals_t = oh_pool.tile([128, 18 * 128], bf16,
                                          tag="vals", name="valst")
                    nc.sync.dma_start(
                        out=vals_t[oo:oo + KROW, :njobs * 128],
                        in_=valsp[g][:, j0 * 128:(j0 + njobs) * 128])
                    oh_t = oh_pool.tile([128, 18 * SUB], fp8,
                                        tag="oh", name="oht")
                    nc.sync.dma_start(
                        out=oh_t[oo:oo + KROW, :njobs * SUB],
                        in_=ohp[g][:, j0 * SUB:(j0 + njobs) * SUB])
                    # PSUM chunks of 4 subtiles (2048 cols); one fused DVE
                    # pass per chunk does scale+merge: t = t*p + ps
                    wjobs = jobs[g][w]
                    for s0 in range(0, nsub, 4):
                        ns = min(4, nsub - s0)
                        ck_lo = s0 * SUB
                        ck_w = sum(sub_geom[i0 + s0 + j][1] for j in range(ns))
                        ps = psum_pool.tile([128, 4 * SUB], f32, tag="ps", name="ps")
                        for (jj, s, wd, st, sp, i, klo) in wjobs:
                            if not (s0 <= s < s0 + ns):
                                continue
                            jb = jj - j0
                            nc.tensor.matmul(
                                out=ps[:, (s - s0) * SUB:(s - s0) * SUB + wd],
                                lhsT=vals_t[oo:oo + KROW, jb * 128:jb * 128 + 128],
                                rhs=oh_t[oo:oo + KROW, jb * SUB:jb * SUB + wd],
                                start=st, stop=sp)
                        nc.vector.scalar_tensor_tensor(
                            out=t[:, ck_lo:ck_lo + ck_w],
                            in0=t[:, ck_lo:ck_lo + ck_w],
                            scalar=p_t[g][:, :1],
                            in1=ps[:, :ck_w],
                            op0=mybir.AluOpType.mult,
                            op1=mybir.AluOpType.add)
                    nc.scalar.dma_start(out=out[g][:, c0w:c0w + ww], in_=t[:, :ww])
    nc.finalize()
    return nc


def kernel(vocab_ds, attns, p_gens, sources, decoder_batch_len):
    T = int(decoder_batch_len)
    in_maps, meta = _host_prep(vocab_ds, attns, p_gens, sources, T)
    nc = _build_nc(meta)

    from concourse.bass_utils import run_bass_kernel_spmd
    res = run_bass_kernel_spmd(nc, in_maps, list(range(N_CORES)))

    rows, V = np.asarray(vocab_ds).shape
    B, BPC, G = meta["B"], meta["BPC"], meta["G"]
    full = np.empty((rows, V), dtype=np.float32)
    for core in range(N_CORES):
        for g in range(G):
            blk = np.asarray(res.results[core][f"out{g}"], dtype=np.float32)
            for half in range(2):
                b = core * BPC + g * 2 + half
                full[b::B] = blk[half * T:(half + 1) * T]
    return full
